# revision 1
# baseline (speedup 1.0000x reference)
"""Dir_Encoder_GCN (2-layer GCNConv + ELU + Softplus) on 8 trn2 NeuronCores.

Strategy (per sharding hint): nodes are dst-sharded across 8 cores; edges are
partitioned by destination shard and sorted by destination. Weights W1/W2 are
replicated. Source-feature tables (dinv-scaled) are exchanged via AllGather.

Math: for each layer, out[d] = dinv[d] * sum_e w_e * (dinv[s_e] * feat[s_e]) + b
with self-loops folded in as ordinary edges of weight 1. The per-edge gather is
an indirect DMA of table rows; the weighted segmented sum is a PE matmul with a
host-prepared scaled one-hot matrix M (M[p, slot_p] = w_p); W is applied after
aggregation via pre^T @ W per 128-slot window (linearity of the aggregation).

Host-side numpy performs only integer index manipulation and data layout
(sorting, window packing, one-hot placement of input edge weights); all
floating-point arithmetic on values happens on-device.
"""

import sys

if "/opt/trn_rl_repo" not in sys.path:
    sys.path.insert(0, "/opt/trn_rl_repo")

import numpy as np

N_NODES = 50000
N_EDGES = 800000
F_IN = 128
F_HID = 128
F_OUT = 64
NCORES = 8
P = 128  # partitions / window slot capacity / edge-tile size


def _pack_windows(dst_local, tw_cap, n_nodes_core):
    """Greedy-pack local nodes into windows of <=128 nodes and <=tw_cap*128
    edges. dst_local: sorted local dst id per edge. Returns list of
    (node_lo, node_hi) per window (node_hi exclusive)."""
    # edges per local node
    counts = np.bincount(dst_local, minlength=n_nodes_core)
    windows = []
    lo = 0
    cur_edges = 0
    hi = 0
    cap = tw_cap * P
    while hi < n_nodes_core:
        c = counts[hi]  # node's total edge count (self-loop included)
        if (hi - lo) >= P or cur_edges + c > cap:
            windows.append((lo, hi))
            lo = hi
            cur_edges = 0
        cur_edges += c
        hi += 1
    windows.append((lo, hi))
    return windows, counts


def build_problem(x, edge_index, edge_weight, W1, b1, W2, b2):
    """Builds the bass program + per-core input maps.

    Returns (nc, in_maps, row_of_node): run the program SPMD on cores 0..7,
    concat the per-core "y_win" outputs, then index with row_of_node to get
    the final [N, F_OUT] output.
    """
    import concourse.bacc as bacc
    import concourse.tile as tile
    from concourse import bass, mybir

    x = np.asarray(x, dtype=np.float32)
    edge_index = np.asarray(edge_index)
    edge_weight = np.asarray(edge_weight, dtype=np.float32)
    W1 = np.asarray(W1, dtype=np.float32)
    b1 = np.asarray(b1, dtype=np.float32)
    W2 = np.asarray(W2, dtype=np.float32)
    b2 = np.asarray(b2, dtype=np.float32)

    n = x.shape[0]
    F_IN = x.shape[1]
    F_HID = W1.shape[1]
    F_OUT = W2.shape[1]

    # ---------------- host-side integer prep ----------------
    # nodes -> cores, contiguous ranges
    per_core_n = (n + NCORES - 1) // NCORES  # 6250
    src = edge_index[0].astype(np.int64)
    dst = edge_index[1].astype(np.int64)

    # fold self-loops in as ordinary edges (weight 1, matching reference)
    src_all = np.concatenate([src, np.arange(n, dtype=np.int64)])
    dst_all = np.concatenate([dst, np.arange(n, dtype=np.int64)])
    w_all = np.concatenate([edge_weight, np.ones(n, dtype=np.float32)])

    order = np.argsort(dst_all, kind="stable")
    s_s = src_all[order]
    d_s = dst_all[order]
    w_s = w_all[order]

    core_edge_bounds = np.searchsorted(
        d_s, [c * per_core_n for c in range(NCORES + 1)]
    )

    # edge tiles per window (17*128 = 2176 edge capacity); raised if any
    # single node's edge count exceeds one window's capacity
    TW = max(17, int(np.ceil((np.bincount(dst_all, minlength=n).max() + 1) / P)))

    # pass 1: per-core window packing to find uniform NWIN
    core_data = []
    nwin_max = 0
    kd_max = 1
    for c in range(NCORES):
        e0, e1 = core_edge_bounds[c], core_edge_bounds[c + 1]
        n_lo = c * per_core_n
        n_hi = min((c + 1) * per_core_n, n)
        n_c = n_hi - n_lo
        dl = (d_s[e0:e1] - n_lo).astype(np.int64)
        windows, counts = _pack_windows(dl, TW, n_c)
        nwin_max = max(nwin_max, len(windows))
        # real (non-self-loop) in-degree for ELL: counts includes self-loop
        kd_max = max(kd_max, int((counts - 1).max(initial=0)))
        core_data.append((e0, e1, n_lo, n_c, dl, windows, counts))

    NWIN = nwin_max
    KD = kd_max
    SH = NWIN * P  # table rows per core (window-slot layout)
    TTOT = NWIN * TW  # edge tiles per core per layer
    VTOT = NCORES * SH

    # pass 2: build per-core arrays
    in_maps = []
    row_of_node = np.zeros(n, dtype=np.int64)  # global table row per node
    node_rows_per_core = []

    # first compute row_of_node for ALL cores (needed for gidx of any core)
    for c in range(NCORES):
        e0, e1, n_lo, n_c, dl, windows, counts = core_data[c]
        rows = np.full(SH, -1, dtype=np.int64)  # local row -> node id
        for wi, (lo, hi) in enumerate(windows):
            ids = np.arange(lo, hi)
            row_of_node[n_lo + ids] = c * SH + wi * P + (ids - lo)
            rows[wi * P : wi * P + (hi - lo)] = n_lo + ids
        node_rows_per_core.append(rows)

    for c in range(NCORES):
        e0, e1, n_lo, n_c, dl, windows, counts = core_data[c]
        e_src = s_s[e0:e1]
        e_w = w_s[e0:e1]
        # edge order is dst-sorted; windows take contiguous edge runs
        node_e0 = np.concatenate([[0], np.cumsum(counts)])  # per local node

        gidx = np.zeros((P, TTOT), dtype=np.int32)
        sw_host = np.zeros((TTOT * P, 2), dtype=np.float32)
        sw_host[:, 0] = -1.0  # pad slots never match iota
        # window-slot layout inputs
        x_win = np.zeros((SH, F_IN), dtype=np.float32)
        wdeg = np.zeros((P, NWIN * KD), dtype=np.float32)

        for wi, (lo, hi) in enumerate(windows):
            ew0, ew1 = node_e0[lo], node_e0[hi]
            cnt = ew1 - ew0
            assert cnt <= TW * P
            wsrc = e_src[ew0:ew1]
            wslot = dl[ew0:ew1] - lo
            ww = e_w[ew0:ew1]
            t0 = wi * TW
            # scatter edges into tiles: edge j -> tile t0 + j//P, partition j%P
            tt = t0 + np.arange(cnt) // P
            pp = np.arange(cnt) % P
            gidx[pp, tt] = row_of_node[wsrc].astype(np.int32)
            sw_host[tt * P + pp, 0] = wslot.astype(np.float32)
            sw_host[tt * P + pp, 1] = ww

            # window-slot node data
            ids = np.arange(lo, hi)
            x_win[wi * P : wi * P + (hi - lo)] = x[n_lo + ids]
            # ELL of real in-edge weights (exclude self-loop weight):
            # edges of node v: e_src slice; self-loop is the one with src==v
            # and w==1 appended last among its dst group (stable sort kept
            # original order: real edges first, then self-loop)
            for v in ids:
                a, b = node_e0[v], node_e0[v + 1]
                # last entry for node v is its self-loop (appended after real
                # edges and stable-sorted)
                realw = e_w[a : b - 1]
                p_ = v - lo
                wdeg[p_, wi * KD : wi * KD + len(realw)] = realw

        in_maps.append(
            {
                "x_win": x_win,
                "wdeg": wdeg,
                "gidx": gidx,
                "sw_stream": sw_host,
                "iotar": np.tile(np.arange(P, dtype=np.float32), (P, 1)),
                "w1": W1,
                "w2": W2,
                "b1b": np.tile(b1[None, :], (P, 1)).astype(np.float32),
                "b2b": np.tile(b2[None, :], (P, 1)).astype(np.float32),
                "ident": np.eye(P, dtype=np.float32),
            }
        )

    # ---------------- device program (uniform across cores) ----------------
    import os

    stage = 9  # debug bisection disabled for production
    nc = bacc.Bacc("TRN2", target_bir_lowering=False, debug=False, num_devices=NCORES)

    x_win_d = nc.dram_tensor("x_win", [SH, F_IN], mybir.dt.float32, kind="ExternalInput")
    wdeg_d = nc.dram_tensor("wdeg", [P, NWIN * KD], mybir.dt.float32, kind="ExternalInput")
    gidx_d = nc.dram_tensor("gidx", [P, TTOT], mybir.dt.int32, kind="ExternalInput")
    sw_d = nc.dram_tensor("sw_stream", [TTOT * P, 2], mybir.dt.float32, kind="ExternalInput")
    iotar_d = nc.dram_tensor("iotar", [P, P], mybir.dt.float32, kind="ExternalInput")
    w1_d = nc.dram_tensor("w1", [F_IN, F_HID], mybir.dt.float32, kind="ExternalInput")
    w2_d = nc.dram_tensor("w2", [F_HID, F_OUT], mybir.dt.float32, kind="ExternalInput")
    b1b_d = nc.dram_tensor("b1b", [P, F_HID], mybir.dt.float32, kind="ExternalInput")
    b2b_d = nc.dram_tensor("b2b", [P, F_OUT], mybir.dt.float32, kind="ExternalInput")
    ident_d = nc.dram_tensor("ident", [P, P], mybir.dt.float32, kind="ExternalInput")
    y_d = nc.dram_tensor("y_win", [SH, F_OUT], mybir.dt.float32, kind="ExternalOutput")

    AF = mybir.ActivationFunctionType
    OP = mybir.AluOpType

    with tile.TileContext(nc) as tc:
        with (
            tc.tile_pool(name="const", bufs=1) as cpool,
            tc.tile_pool(name="gpool", bufs=24) as gpool,
            tc.tile_pool(name="mpool", bufs=3) as mpool,
            tc.tile_pool(name="mbuild", bufs=12) as mbuild,
            tc.tile_pool(name="post", bufs=3) as post,
            tc.tile_pool(name="pacc", bufs=2, space="PSUM") as pacc,
            tc.tile_pool(name="pmisc", bufs=2, space="PSUM") as pmisc,
            tc.tile_pool(name="dram", bufs=1, space="DRAM") as dpool,
        ):
            # constants
            w1_t = cpool.tile([F_IN, F_HID], mybir.dt.float32)
            nc.sync.dma_start(out=w1_t[:], in_=w1_d[:])
            w2_t = cpool.tile([F_HID, F_OUT], mybir.dt.float32)
            nc.sync.dma_start(out=w2_t[:], in_=w2_d[:])
            b1b_t = cpool.tile([P, F_HID], mybir.dt.float32)
            nc.sync.dma_start(out=b1b_t[:], in_=b1b_d[:])
            b2b_t = cpool.tile([P, F_OUT], mybir.dt.float32)
            nc.sync.dma_start(out=b2b_t[:], in_=b2b_d[:])
            ident_t = cpool.tile([P, P], mybir.dt.float32)
            nc.sync.dma_start(out=ident_t[:], in_=ident_d[:])
            iotar_t = cpool.tile([P, P], mybir.dt.float32)
            nc.sync.dma_start(out=iotar_t[:], in_=iotar_d[:])
            gidx_t = cpool.tile([P, TTOT], mybir.dt.int32)
            nc.sync.dma_start(out=gidx_t[:], in_=gidx_d[:])

            # ---- degree -> dinv [P, NWIN] ----
            wdeg_t = cpool.tile([P, NWIN * KD], mybir.dt.float32)
            nc.sync.dma_start(out=wdeg_t[:], in_=wdeg_d[:])
            dsum_t = cpool.tile([P, NWIN], mybir.dt.float32)
            nc.vector.tensor_reduce(
                out=dsum_t[:],
                in_=wdeg_t[:].rearrange("p (w k) -> p w k", k=KD),
                axis=mybir.AxisListType.X,
                op=OP.add,
            )
            # deg = sum + 1 (self-loop); dinv = sqrt(1/deg)
            recip_t = cpool.tile([P, NWIN], mybir.dt.float32)
            nc.vector.tensor_scalar_add(out=dsum_t[:], in0=dsum_t[:], scalar1=1.0)
            nc.vector.reciprocal(out=recip_t[:], in_=dsum_t[:])
            dinv_t = cpool.tile([P, NWIN], mybir.dt.float32)
            nc.scalar.activation(out=dinv_t[:], in_=recip_t[:], func=AF.Sqrt)

            # ---- xsc table build + allgather ----
            def dummy_out():
                dummy = post.tile([P, F_OUT], mybir.dt.float32, tag="yf")
                nc.scalar.activation(out=dummy[:], in_=b2b_t[:], func=AF.Copy)
                nc.sync.dma_start(out=y_d[0:P, :], in_=dummy[:])

            xsc_shard = dpool.tile([SH, F_IN], mybir.dt.float32)
            xsc_full = dpool.tile([VTOT, F_IN], mybir.dt.float32, addr_space="Shared")
            for wi in range(NWIN):
                xw_t = post.tile([P, F_IN], mybir.dt.float32, tag="xw")
                nc.sync.dma_start(
                    out=xw_t[:], in_=x_win_d[wi * P : (wi + 1) * P, :]
                )
                xs_t = post.tile([P, F_IN], mybir.dt.float32, tag="xs")
                nc.vector.tensor_scalar(
                    out=xs_t[:], in0=xw_t[:],
                    scalar1=dinv_t[:, wi : wi + 1], scalar2=None, op0=OP.mult,
                )
                nc.sync.dma_start(
                    out=xsc_shard[wi * P : (wi + 1) * P, :], in_=xs_t[:]
                )
            if stage >= 2:
                nc.gpsimd.collective_compute(
                    "AllGather",
                    OP.bypass,
                    replica_groups=[list(range(NCORES))],
                    ins=[xsc_shard.opt()],
                    outs=[xsc_full.opt()],
                )

            hs_shard = dpool.tile([SH, F_HID], mybir.dt.float32)
            hs_full = dpool.tile([VTOT, F_HID], mybir.dt.float32, addr_space="Shared")

            def layer(table_full, fdim, w_t, bb_t, out_write):
                """One GCN layer. out_write(wi, tile[P, fout]) stores result."""
                for wi in range(NWIN):
                    swwin_t = mpool.tile([P, TW * 2], mybir.dt.float32, tag="swwin")
                    nc.sync.dma_start(
                        out=swwin_t[:].rearrange("p (t c) -> p t c", c=2),
                        in_=sw_d[wi * TW * P : (wi + 1) * TW * P, :].rearrange(
                            "(t p) c -> p t c", p=P
                        ),
                    )
                    acc = pacc.tile([P, fdim], mybir.dt.float32, space="PSUM", tag="acc")
                    for t in range(TW):
                        ti = wi * TW + t
                        g_t = gpool.tile([P, fdim], mybir.dt.float32, tag="g")
                        nc.gpsimd.indirect_dma_start(
                            out=g_t[:],
                            out_offset=None,
                            in_=table_full.opt(),
                            in_offset=bass.IndirectOffsetOnAxis(
                                ap=gidx_t[:, ti : ti + 1], axis=0
                            ),
                        )
                        m_t = mbuild.tile([P, P], mybir.dt.float32, tag="mt")
                        nc.vector.tensor_scalar(
                            out=m_t[:], in0=iotar_t[:],
                            scalar1=swwin_t[:, 2 * t : 2 * t + 1],
                            scalar2=swwin_t[:, 2 * t + 1 : 2 * t + 2],
                            op0=OP.is_equal, op1=OP.mult,
                        )
                        nc.tensor.matmul(
                            out=acc[:],
                            lhsT=m_t[:],
                            rhs=g_t[:],
                            start=(t == 0),
                            stop=(t == TW - 1),
                        )
                    # pre = acc * dinv (per-slot) ; transpose; @W ; activations
                    pre_t = post.tile([P, fdim], mybir.dt.float32, tag="pre")
                    nc.vector.tensor_scalar(
                        out=pre_t[:], in0=acc[:],
                        scalar1=dinv_t[:, wi : wi + 1], scalar2=None, op0=OP.mult,
                    )
                    preT_ps = pmisc.tile([P, fdim], mybir.dt.float32, space="PSUM", tag="preT")
                    nc.tensor.transpose(
                        out=preT_ps[:], in_=pre_t[:], identity=ident_t[:]
                    )
                    preT_t = post.tile([P, fdim], mybir.dt.float32, tag="preT_sb")
                    nc.scalar.copy(out=preT_t[:], in_=preT_ps[:])
                    fout = w_t.shape[1]
                    h_ps = pmisc.tile([P, fout], mybir.dt.float32, space="PSUM", tag="h")
                    nc.tensor.matmul(
                        out=h_ps[:], lhsT=preT_t[:], rhs=w_t[:],
                        start=True, stop=True,
                    )
                    out_write(wi, h_ps)

            # ---- layer 1: table xsc, act = elu, write hs ----
            def l1_out(wi, h_ps):
                hb_t = post.tile([P, F_HID], mybir.dt.float32, tag="hb")
                nc.vector.tensor_add(out=hb_t[:], in0=h_ps[:], in1=b1b_t[:])
                mn_t = post.tile([P, F_HID], mybir.dt.float32, tag="mn")
                nc.vector.tensor_scalar_min(out=mn_t[:], in0=hb_t[:], scalar1=0.0)
                ex_t = post.tile([P, F_HID], mybir.dt.float32, tag="ex")
                nc.scalar.activation(out=ex_t[:], in_=mn_t[:], func=AF.Exp)
                rl_t = post.tile([P, F_HID], mybir.dt.float32, tag="rl")
                nc.vector.tensor_scalar_max(out=rl_t[:], in0=hb_t[:], scalar1=0.0)
                h_t = post.tile([P, F_HID], mybir.dt.float32, tag="hf")
                # (relu - 1) + exp(min(x,0)) = elu
                nc.vector.scalar_tensor_tensor(
                    out=h_t[:], in0=rl_t[:], scalar=-1.0, in1=ex_t[:],
                    op0=OP.add, op1=OP.add,
                )
                hsv_t = post.tile([P, F_HID], mybir.dt.float32, tag="hsv")
                nc.vector.tensor_scalar(
                    out=hsv_t[:], in0=h_t[:],
                    scalar1=dinv_t[:, wi : wi + 1], scalar2=None, op0=OP.mult,
                )
                nc.sync.dma_start(
                    out=hs_shard[wi * P : (wi + 1) * P, :], in_=hsv_t[:]
                )

            if stage >= 3:
                layer(xsc_full, F_IN, w1_t, b1b_t, l1_out)

            if stage >= 4:
                nc.gpsimd.collective_compute(
                    "AllGather",
                    OP.bypass,
                    replica_groups=[list(range(NCORES))],
                    ins=[hs_shard.opt()],
                    outs=[hs_full.opt()],
                )

            # ---- layer 2: table hs, act = softplus + 1e-4 ----
            def l2_out(wi, y_ps):
                # softplus(x) = max(x,0) + ln(1 + exp(-|x|)), then + 1e-4
                yb_t = post.tile([P, F_OUT], mybir.dt.float32, tag="yb")
                nc.vector.tensor_add(out=yb_t[:], in0=y_ps[:], in1=b2b_t[:])
                na_t = post.tile([P, F_OUT], mybir.dt.float32, tag="na")
                nc.vector.scalar_tensor_tensor(
                    out=na_t[:], in0=yb_t[:], scalar=-1.0, in1=yb_t[:],
                    op0=OP.mult, op1=OP.min,
                )
                ex2_t = post.tile([P, F_OUT], mybir.dt.float32, tag="ex2")
                nc.scalar.activation(out=ex2_t[:], in_=na_t[:], func=AF.Exp)
                ln_t = post.tile([P, F_OUT], mybir.dt.float32, tag="ln")
                nc.scalar.activation(out=ln_t[:], in_=ex2_t[:], func=AF.Ln, bias=1.0)
                sp_t = post.tile([P, F_OUT], mybir.dt.float32, tag="sp")
                nc.vector.scalar_tensor_tensor(
                    out=sp_t[:], in0=yb_t[:], scalar=0.0, in1=ln_t[:],
                    op0=OP.max, op1=OP.add,
                )
                yf_t = post.tile([P, F_OUT], mybir.dt.float32, tag="yf")
                nc.vector.tensor_scalar_add(out=yf_t[:], in0=sp_t[:], scalar1=1e-4)
                nc.sync.dma_start(
                    out=y_d[wi * P : (wi + 1) * P, :], in_=yf_t[:]
                )

            if stage >= 5:
                layer(hs_full, F_HID, w2_t, b2b_t, l2_out)
            else:
                dummy_out()

    nc.compile()
    return nc, in_maps, row_of_node


def kernel(x, edge_index, edge_weight, W1, b1, W2, b2):
    import time

    from concourse.bass_utils import run_bass_kernel_spmd

    nc, in_maps, row_of_node = build_problem(
        x, edge_index, edge_weight, W1, b1, W2, b2
    )
    # Device execution occasionally hits a transient NRT failure on the first
    # run of a fresh NEFF; the mesh recovers after a pause, so retry.
    last_err = None
    for attempt in range(3):
        try:
            res = run_bass_kernel_spmd(nc, in_maps, core_ids=list(range(NCORES)))
            break
        except Exception as e:
            last_err = e
            try:
                import jax

                jax.clear_caches()
            except Exception:
                pass
            time.sleep(30 * (attempt + 1))
    else:
        raise last_err
    y_full = np.concatenate([res.results[c]["y_win"] for c in range(NCORES)], axis=0)
    out = y_full[row_of_node]  # [n, F_OUT]
    return out.astype(np.float32)


if __name__ == "__main__":
    # quick shape smoke with random data
    rng = np.random.default_rng(0)
    x = rng.standard_normal((N_NODES, F_IN)).astype(np.float32)
    ei = rng.integers(0, N_NODES, size=(2, N_EDGES)).astype(np.int64)
    ew = rng.random(N_EDGES).astype(np.float32)
    W1 = rng.standard_normal((F_IN, F_HID)).astype(np.float32) * 0.09
    W2 = rng.standard_normal((F_HID, F_OUT)).astype(np.float32) * 0.09
    y = kernel(x, ei, ew, W1, np.zeros(F_HID, np.float32), W2, np.zeros(F_OUT, np.float32))
    print(y.shape, y.dtype, np.isfinite(y).all())



# revision 10
# speedup vs baseline: 1.5762x; 1.5762x over previous
"""Dir_Encoder_GCN (2-layer GCNConv + ELU + Softplus) on 8 trn2 NeuronCores.

Strategy: nodes dst-sharded across 8 cores; edges partitioned by destination
shard and sorted by destination. Weights W1/W2 replicated. Per-layer node
tables are pre-transformed on device (z1 = (dinv*x)@W1, z2 = (dinv*h)@W2) so
aggregation gathers already-transformed rows, then AllGathered so every core
can gather any source row.

Aggregation: edges stream in 1024-edge groups fetched with ONE
InstDMAGatherAnt each (int16 indices, biased at table row 32768 so signed
indices span the whole 50176-row table). Each 128-edge tile is reduced with
a PE matmul against a one-hot matrix M (M[p, slot_p] = w_p) built on DVE from
a resident (slot, weight) stream; per-window PSUM accumulates across tiles.
dinv[dst] is applied post-aggregation, dinv[src] is folded into the tables.

Host-side numpy performs only integer index manipulation and data layout
(sorting, window/tile packing, int16 index streams, one-hot placement of
input edge weights); all floating-point arithmetic on values happens
on-device.
"""

import os
import sys

if "/opt/trn_rl_repo" not in sys.path:
    sys.path.insert(0, "/opt/trn_rl_repo")

import numpy as np

N_NODES = 50000
N_EDGES = 800000
F_IN = 128
F_HID = 128
F_OUT = 64
NCORES = 8
P = 128
GRP = 8          # tiles per gather group (8*128 = 1024 idx, the HW cap)
IDX_BASE = 32768  # int16 index bias into the global table


def build_problem(x, edge_index, edge_weight, W1, b1, W2, b2):
    """Builds the bass program + per-core input maps.

    Returns (nc, in_maps, row_of_node): run the program SPMD on cores 0..7,
    concat the per-core "y_win" outputs, then index with row_of_node to get
    the final [N, F_OUT] output.
    """
    import concourse.bacc as bacc
    import concourse.tile as tile
    from concourse import bass, mybir

    x = np.asarray(x, dtype=np.float32)
    edge_index = np.asarray(edge_index)
    edge_weight = np.asarray(edge_weight, dtype=np.float32)
    W1 = np.asarray(W1, dtype=np.float32)
    b1 = np.asarray(b1, dtype=np.float32)
    W2 = np.asarray(W2, dtype=np.float32)
    b2 = np.asarray(b2, dtype=np.float32)

    n = x.shape[0]

    # ---------------- host-side integer prep ----------------
    per_core_n = (n + NCORES - 1) // NCORES  # 6250
    NWIN = (per_core_n + P - 1) // P         # 49
    SH = NWIN * P                            # 6272 table rows per core
    VTOT = NCORES * SH                       # 50176

    src = edge_index[0].astype(np.int64)
    dst = edge_index[1].astype(np.int64)

    # fold self-loops in as ordinary edges (weight 1, matching reference)
    src_all = np.concatenate([src, np.arange(n, dtype=np.int64)])
    dst_all = np.concatenate([dst, np.arange(n, dtype=np.int64)])
    w_all = np.concatenate([edge_weight, np.ones(n, dtype=np.float32)])

    order = np.argsort(dst_all, kind="stable")
    s_s = src_all[order]
    d_s = dst_all[order]
    w_s = w_all[order]

    core_edge_bounds = np.searchsorted(
        d_s, [c * per_core_n for c in range(NCORES + 1)]
    )

    # global node -> table row
    node_ids = np.arange(n, dtype=np.int64)
    core_of = node_ids // per_core_n
    local = node_ids - core_of * per_core_n
    row_of_node = core_of * SH + local  # window w = local//P, slot = local%P

    # per-(core, window) edge counts -> uniform tiles-per-window
    counts_cw = np.zeros((NCORES, NWIN), dtype=np.int64)
    core_data = []
    for c in range(NCORES):
        e0, e1 = core_edge_bounds[c], core_edge_bounds[c + 1]
        dl = (d_s[e0:e1] - c * per_core_n).astype(np.int64)
        cnt = np.bincount(dl // P, minlength=NWIN)
        counts_cw[c] = cnt
        core_data.append((e0, e1, dl))
    tiles_w = np.maximum(1, (counts_cw.max(axis=0) + P - 1) // P)  # [NWIN]
    off_w = np.concatenate([[0], np.cumsum(tiles_w)])
    NT = int(off_w[-1])                       # total tiles per core
    NGRP = (NT + GRP - 1) // GRP

    # per-core ELL of real (non-self-loop) in-edge weights for degrees
    KD = 1
    for c in range(NCORES):
        e0, e1, dl = core_data[c]
        cnt = np.bincount(dl, minlength=per_core_n)
        KD = max(KD, int(cnt.max(initial=1)) - 1)

    in_maps = []
    for c in range(NCORES):
        e0, e1, dl = core_data[c]
        e_src = s_s[e0:e1]
        e_w = w_s[e0:e1]
        node_cnt = np.bincount(dl, minlength=per_core_n)
        node_e0 = np.concatenate([[0], np.cumsum(node_cnt)])

        # streams over the uniform tile layout
        idx16 = np.zeros(NT * P, dtype=np.int16)          # pad -> row IDX_BASE
        slot_st = np.full(NT * P, -1.0, dtype=np.float32)  # pad slot -1
        w_st = np.zeros(NT * P, dtype=np.float32)

        win_of_edge = dl // P
        wslot = dl % P
        # position within window's edge run (edges are dst-sorted, windows
        # take contiguous runs)
        win_e0 = np.concatenate(
            [[0], np.cumsum(counts_cw[c])]
        )  # first edge index per window
        j_in_win = np.arange(e1 - e0) - win_e0[win_of_edge]
        pos = (off_w[win_of_edge] + j_in_win // P) * P + (j_in_win % P)
        idx16[pos] = (row_of_node[e_src] - IDX_BASE).astype(np.int16)
        slot_st[pos] = wslot.astype(np.float32)
        w_st[pos] = e_w

        # ucode trims TRAILING negative indices of each gather (treats them
        # as padding), so the last index of every group must be >= 0: swap
        # within the group's final tile (tile assignment, slot and weight all
        # move together, which the one-hot M absorbs).
        for g in range(NGRP):
            L = min((g + 1) * GRP, NT) * P - 1
            if idx16[L] < 0:
                T = L // P
                cand = np.where(idx16[T * P : (T + 1) * P] >= 0)[0]
                assert len(cand) > 0, "group-final tile has no non-negative idx"
                q = T * P + int(cand[0])
                for arr in (idx16, slot_st, w_st):
                    arr[L], arr[q] = arr[q], arr[L]

        # wrapped int16 gather indices: group g, linear i -> [i%16, g*64+i//16]
        gidx_w = np.zeros((16, NGRP * (GRP * P // 16)), dtype=np.int16)
        for g in range(NGRP):
            t0, t1 = g * GRP, min((g + 1) * GRP, NT)
            seg = idx16[t0 * P : t1 * P]
            ncols = len(seg) // 16
            gidx_w[:, g * 64 : g * 64 + ncols] = seg.reshape(ncols, 16).T
        gidx_full = np.tile(gidx_w, (8, 1))  # [128, NGRP*64]

        # sw stream [128, NT*2]: [p, 2t] = slot, [p, 2t+1] = w
        sw_host = np.empty((P, NT * 2), dtype=np.float32)
        sw_host[:, 0::2] = slot_st.reshape(NT, P).T
        sw_host[:, 1::2] = w_st.reshape(NT, P).T

        # window-slot node data
        x_win = np.zeros((SH, F_IN), dtype=np.float32)
        n_c = min((c + 1) * per_core_n, n) - c * per_core_n
        x_win[:n_c] = x[c * per_core_n : c * per_core_n + n_c]

        # ELL of real in-edge weights (self-loop is last in each node's run)
        wdeg = np.zeros((P, NWIN * KD), dtype=np.float32)
        for v in range(n_c):
            a, b = node_e0[v], node_e0[v + 1]
            realw = e_w[a : b - 1]
            wi, sl = v // P, v % P
            wdeg[sl, wi * KD : wi * KD + len(realw)] = realw

        in_maps.append(
            {
                "x_win": x_win,
                "wdeg": wdeg,
                "gidx": gidx_full,
                "sw_stream": sw_host,
                "iotar": np.tile(np.arange(P, dtype=np.float32), (P, 1)),
                "w1": W1,
                "w2": W2,
                "b1b": np.tile(b1[None, :], (P, 1)).astype(np.float32),
                "b2b": np.tile(b2[None, :], (P, 1)).astype(np.float32),
                "ident": np.eye(P, dtype=np.float32),
            }
        )

    # ---------------- device program (uniform across cores) ----------------
    stage = int(os.environ.get("BASS_STAGE", "9"))
    nc = bacc.Bacc("TRN2", target_bir_lowering=False, debug=False, num_devices=NCORES)

    x_win_d = nc.dram_tensor("x_win", [SH, F_IN], mybir.dt.float32, kind="ExternalInput")
    wdeg_d = nc.dram_tensor("wdeg", [P, NWIN * KD], mybir.dt.float32, kind="ExternalInput")
    gidx_d = nc.dram_tensor("gidx", [P, NGRP * 64], mybir.dt.int16, kind="ExternalInput")
    sw_d = nc.dram_tensor("sw_stream", [P, NT * 2], mybir.dt.float32, kind="ExternalInput")
    iotar_d = nc.dram_tensor("iotar", [P, P], mybir.dt.float32, kind="ExternalInput")
    w1_d = nc.dram_tensor("w1", [F_IN, F_HID], mybir.dt.float32, kind="ExternalInput")
    w2_d = nc.dram_tensor("w2", [F_HID, F_OUT], mybir.dt.float32, kind="ExternalInput")
    b1b_d = nc.dram_tensor("b1b", [P, F_HID], mybir.dt.float32, kind="ExternalInput")
    b2b_d = nc.dram_tensor("b2b", [P, F_OUT], mybir.dt.float32, kind="ExternalInput")
    ident_d = nc.dram_tensor("ident", [P, P], mybir.dt.float32, kind="ExternalInput")
    y_d = nc.dram_tensor("y_win", [SH, F_OUT], mybir.dt.float32, kind="ExternalOutput")
    debug = os.environ.get("BASS_DEBUG", "0") == "1"
    if debug:
        dbg_z1_d = nc.dram_tensor(
            "dbg_z1", [VTOT, F_HID], mybir.dt.float32, kind="ExternalOutput"
        )
        dbg_h_d = nc.dram_tensor(
            "dbg_h", [SH, F_HID], mybir.dt.float32, kind="ExternalOutput"
        )
        dbg_g_d = nc.dram_tensor(
            "dbg_g", [P, 4 * GRP * F_HID], mybir.dt.float32, kind="ExternalOutput"
        )

    AF = mybir.ActivationFunctionType
    OP = mybir.AluOpType

    with tile.TileContext(nc) as tc:
        with (
            tc.tile_pool(name="const", bufs=1) as cpool,
            tc.tile_pool(name="gpool", bufs=4) as gpool,
            tc.tile_pool(name="mbuild", bufs=8) as mbuild,
            tc.tile_pool(name="post", bufs=3) as post,
            tc.tile_pool(name="pacc", bufs=2, space="PSUM") as pacc,
            tc.tile_pool(name="pmisc", bufs=2, space="PSUM") as pmisc,
            tc.tile_pool(name="dram", bufs=1, space="DRAM") as dpool,
        ):
            # constants
            w1_t = cpool.tile([F_IN, F_HID], mybir.dt.float32)
            nc.sync.dma_start(out=w1_t[:], in_=w1_d[:])
            w2_t = cpool.tile([F_HID, F_OUT], mybir.dt.float32)
            nc.sync.dma_start(out=w2_t[:], in_=w2_d[:])
            b1b_t = cpool.tile([P, F_HID], mybir.dt.float32)
            nc.sync.dma_start(out=b1b_t[:], in_=b1b_d[:])
            b2b_t = cpool.tile([P, F_OUT], mybir.dt.float32)
            nc.sync.dma_start(out=b2b_t[:], in_=b2b_d[:])
            ident_t = cpool.tile([P, P], mybir.dt.float32)
            nc.sync.dma_start(out=ident_t[:], in_=ident_d[:])
            iotar_t = cpool.tile([P, P], mybir.dt.float32)
            nc.sync.dma_start(out=iotar_t[:], in_=iotar_d[:])
            gidx_t = cpool.tile([P, NGRP * 64], mybir.dt.int16)
            nc.sync.dma_start(out=gidx_t[:], in_=gidx_d[:])
            sw_t = cpool.tile([P, NT * 2], mybir.dt.float32)
            nc.sync.dma_start(out=sw_t[:], in_=sw_d[:])

            # ---- degree -> dinv [P, NWIN] ----
            wdeg_t = cpool.tile([P, NWIN * KD], mybir.dt.float32)
            nc.sync.dma_start(out=wdeg_t[:], in_=wdeg_d[:])
            dsum_t = cpool.tile([P, NWIN], mybir.dt.float32)
            nc.vector.tensor_reduce(
                out=dsum_t[:],
                in_=wdeg_t[:].rearrange("p (w k) -> p w k", k=KD),
                axis=mybir.AxisListType.X,
                op=OP.add,
            )
            recip_t = cpool.tile([P, NWIN], mybir.dt.float32)
            nc.vector.tensor_scalar_add(out=dsum_t[:], in0=dsum_t[:], scalar1=1.0)
            nc.vector.reciprocal(out=recip_t[:], in_=dsum_t[:])
            dinv_t = cpool.tile([P, NWIN], mybir.dt.float32)
            nc.scalar.activation(out=dinv_t[:], in_=recip_t[:], func=AF.Sqrt)

            def dummy_out():
                dummy = post.tile([P, F_OUT], mybir.dt.float32, tag="yf")
                nc.scalar.activation(out=dummy[:], in_=b2b_t[:], func=AF.Copy)
                nc.sync.dma_start(out=y_d[0:P, :], in_=dummy[:])

            # ---- z1 = (dinv * x) @ W1, shard + allgather ----
            z1_shard = dpool.tile([SH, F_HID], mybir.dt.float32)
            z1_full = dpool.tile([VTOT, F_HID], mybir.dt.float32, addr_space="Shared")
            for wi in range(NWIN):
                xw_t = post.tile([P, F_IN], mybir.dt.float32, tag="xw")
                nc.sync.dma_start(out=xw_t[:], in_=x_win_d[wi * P : (wi + 1) * P, :])
                xs_t = post.tile([P, F_IN], mybir.dt.float32, tag="xs")
                nc.vector.tensor_scalar(
                    out=xs_t[:], in0=xw_t[:],
                    scalar1=dinv_t[:, wi : wi + 1], scalar2=None, op0=OP.mult,
                )
                xT_ps = pmisc.tile([P, F_IN], mybir.dt.float32, space="PSUM", tag="tp")
                nc.tensor.transpose(out=xT_ps[:], in_=xs_t[:], identity=ident_t[:])
                xT_t = post.tile([P, F_IN], mybir.dt.float32, tag="xT_sb")
                nc.scalar.copy(out=xT_t[:], in_=xT_ps[:])
                z1_ps = pmisc.tile([P, F_HID], mybir.dt.float32, space="PSUM", tag="zmm")
                nc.tensor.matmul(
                    out=z1_ps[:], lhsT=xT_t[:], rhs=w1_t[:], start=True, stop=True
                )
                z1_t = post.tile([P, F_HID], mybir.dt.float32, tag="z1_sb")
                nc.scalar.copy(out=z1_t[:], in_=z1_ps[:])
                nc.sync.dma_start(
                    out=z1_shard[wi * P : (wi + 1) * P, :], in_=z1_t[:]
                )

            if stage >= 2:
                nc.gpsimd.collective_compute(
                    "AllGather",
                    OP.bypass,
                    replica_groups=[list(range(NCORES))],
                    ins=[z1_shard.opt()],
                    outs=[z1_full.opt()],
                )

            if debug and stage >= 2:
                for bi in range(0, VTOT, SH):
                    db_t = post.tile([P, F_HID], mybir.dt.float32, tag="dbg")
                    for wj in range(bi, bi + SH, P):
                        db_t = post.tile([P, F_HID], mybir.dt.float32, tag="dbg")
                        nc.sync.dma_start(out=db_t[:], in_=z1_full[wj : wj + P, :])
                        nc.sync.dma_start(out=dbg_z1_d[wj : wj + P, :], in_=db_t[:])

            z2_shard = dpool.tile([SH, F_OUT], mybir.dt.float32)
            z2_full = dpool.tile([VTOT, F_OUT], mybir.dt.float32, addr_space="Shared")

            def aggregate(table_full, fdim, post_fn):
                """Stream edge groups; per tile one M-build + one matmul into
                the owning window's PSUM acc; post_fn(wi, acc_ps) on window
                completion."""
                acc = None
                cur_win = -1
                for g in range(NGRP):
                    t0, t1 = g * GRP, min((g + 1) * GRP, NT)
                    ntile = t1 - t0
                    gbuf = gpool.tile([P, GRP * fdim], mybir.dt.float32, tag="gbuf")
                    nc.gpsimd.dma_gather(
                        out_ap=gbuf[:, : ntile * fdim].rearrange(
                            "p (t f) -> p t f", f=fdim
                        ),
                        in_ap=table_full[IDX_BASE:VTOT, :],
                        idxs_ap=gidx_t[:, g * 64 : g * 64 + ntile * 8],
                        num_idxs=ntile * P,
                        num_idxs_reg=ntile * P,
                        elem_size=fdim,
                    )
                    if debug and fdim == F_HID and g < 4:
                        nc.sync.dma_start(
                            out=dbg_g_d[:, g * GRP * F_HID : (g + 1) * GRP * F_HID],
                            in_=gbuf[:, : GRP * F_HID],
                        )
                    for ti in range(t0, t1):
                        wi = int(np.searchsorted(off_w, ti, side="right") - 1)
                        if wi != cur_win:
                            cur_win = wi
                            acc = pacc.tile(
                                [P, fdim], mybir.dt.float32, space="PSUM", tag="acc"
                            )
                        m_t = mbuild.tile([P, P], mybir.dt.float32, tag="mt")
                        nc.vector.tensor_scalar(
                            out=m_t[:], in0=iotar_t[:],
                            scalar1=sw_t[:, 2 * ti : 2 * ti + 1],
                            scalar2=sw_t[:, 2 * ti + 1 : 2 * ti + 2],
                            op0=OP.is_equal, op1=OP.mult,
                        )
                        tloc = ti - t0
                        nc.tensor.matmul(
                            out=acc[:],
                            lhsT=m_t[:],
                            rhs=gbuf[:, tloc * fdim : (tloc + 1) * fdim],
                            start=(ti == off_w[wi]),
                            stop=(ti == off_w[wi + 1] - 1),
                        )
                        if ti == off_w[wi + 1] - 1:
                            post_fn(wi, acc)

            # ---- layer 1 post: h = elu(acc*dinv + b1); z2 = (dinv*h) @ W2 ----
            def l1_post(wi, acc_ps):
                pre_t = post.tile([P, F_HID], mybir.dt.float32, tag="pre")
                nc.vector.tensor_scalar(
                    out=pre_t[:], in0=acc_ps[:],
                    scalar1=dinv_t[:, wi : wi + 1], scalar2=None, op0=OP.mult,
                )
                hb_t = post.tile([P, F_HID], mybir.dt.float32, tag="hb")
                nc.vector.tensor_add(out=hb_t[:], in0=pre_t[:], in1=b1b_t[:])
                mn_t = post.tile([P, F_HID], mybir.dt.float32, tag="mn")
                nc.vector.tensor_scalar_min(out=mn_t[:], in0=hb_t[:], scalar1=0.0)
                ex_t = post.tile([P, F_HID], mybir.dt.float32, tag="ex")
                nc.scalar.activation(out=ex_t[:], in_=mn_t[:], func=AF.Exp)
                rl_t = post.tile([P, F_HID], mybir.dt.float32, tag="rl")
                nc.vector.tensor_scalar_max(out=rl_t[:], in0=hb_t[:], scalar1=0.0)
                h_t = post.tile([P, F_HID], mybir.dt.float32, tag="hf")
                nc.vector.scalar_tensor_tensor(
                    out=h_t[:], in0=rl_t[:], scalar=-1.0, in1=ex_t[:],
                    op0=OP.add, op1=OP.add,
                )
                hsv_t = post.tile([P, F_HID], mybir.dt.float32, tag="hsv")
                nc.vector.tensor_scalar(
                    out=hsv_t[:], in0=h_t[:],
                    scalar1=dinv_t[:, wi : wi + 1], scalar2=None, op0=OP.mult,
                )
                if debug:
                    nc.sync.dma_start(
                        out=dbg_h_d[wi * P : (wi + 1) * P, :], in_=hsv_t[:]
                    )
                hT_ps = pmisc.tile([P, F_HID], mybir.dt.float32, space="PSUM", tag="tp")
                nc.tensor.transpose(out=hT_ps[:], in_=hsv_t[:], identity=ident_t[:])
                hT_t = post.tile([P, F_HID], mybir.dt.float32, tag="hT_sb")
                nc.scalar.copy(out=hT_t[:], in_=hT_ps[:])
                z2_ps = pmisc.tile([P, F_OUT], mybir.dt.float32, space="PSUM", tag="zmm")
                nc.tensor.matmul(
                    out=z2_ps[:], lhsT=hT_t[:], rhs=w2_t[:], start=True, stop=True
                )
                z2_t = post.tile([P, F_OUT], mybir.dt.float32, tag="z2_sb")
                nc.scalar.copy(out=z2_t[:], in_=z2_ps[:])
                nc.sync.dma_start(
                    out=z2_shard[wi * P : (wi + 1) * P, :], in_=z2_t[:]
                )

            if stage >= 3:
                aggregate(z1_full, F_HID, l1_post)

            if stage >= 4:
                nc.gpsimd.collective_compute(
                    "AllGather",
                    OP.bypass,
                    replica_groups=[list(range(NCORES))],
                    ins=[z2_shard.opt()],
                    outs=[z2_full.opt()],
                )

            # ---- layer 2 post: y = softplus(acc*dinv + b2) + 1e-4 ----
            def l2_post(wi, acc_ps):
                yb_t = post.tile([P, F_OUT], mybir.dt.float32, tag="yb")
                nc.vector.tensor_scalar(
                    out=yb_t[:], in0=acc_ps[:],
                    scalar1=dinv_t[:, wi : wi + 1], scalar2=None, op0=OP.mult,
                )
                nc.vector.tensor_add(out=yb_t[:], in0=yb_t[:], in1=b2b_t[:])
                na_t = post.tile([P, F_OUT], mybir.dt.float32, tag="na")
                nc.vector.scalar_tensor_tensor(
                    out=na_t[:], in0=yb_t[:], scalar=-1.0, in1=yb_t[:],
                    op0=OP.mult, op1=OP.min,
                )
                ex2_t = post.tile([P, F_OUT], mybir.dt.float32, tag="ex2")
                nc.scalar.activation(out=ex2_t[:], in_=na_t[:], func=AF.Exp)
                ln_t = post.tile([P, F_OUT], mybir.dt.float32, tag="ln")
                nc.scalar.activation(out=ln_t[:], in_=ex2_t[:], func=AF.Ln, bias=1.0)
                sp_t = post.tile([P, F_OUT], mybir.dt.float32, tag="sp")
                nc.vector.scalar_tensor_tensor(
                    out=sp_t[:], in0=yb_t[:], scalar=0.0, in1=ln_t[:],
                    op0=OP.max, op1=OP.add,
                )
                yf_t = post.tile([P, F_OUT], mybir.dt.float32, tag="yf")
                nc.vector.tensor_scalar_add(out=yf_t[:], in0=sp_t[:], scalar1=1e-4)
                nc.sync.dma_start(out=y_d[wi * P : (wi + 1) * P, :], in_=yf_t[:])

            if stage >= 5:
                aggregate(z2_full, F_OUT, l2_post)
            else:
                dummy_out()

    nc.compile()
    return nc, in_maps, row_of_node


def kernel(x, edge_index, edge_weight, W1, b1, W2, b2):
    import time

    from concourse.bass_utils import run_bass_kernel_spmd

    nc, in_maps, row_of_node = build_problem(
        x, edge_index, edge_weight, W1, b1, W2, b2
    )
    last_err = None
    for attempt in range(3):
        try:
            res = run_bass_kernel_spmd(nc, in_maps, core_ids=list(range(NCORES)))
            break
        except Exception as e:
            last_err = e
            try:
                import jax

                jax.clear_caches()
            except Exception:
                pass
            time.sleep(30 * (attempt + 1))
    else:
        raise last_err
    y_full = np.concatenate([res.results[c]["y_win"] for c in range(NCORES)], axis=0)
    out = y_full[row_of_node]
    return out.astype(np.float32)


if __name__ == "__main__":
    rng = np.random.default_rng(0)
    x = rng.standard_normal((N_NODES, F_IN)).astype(np.float32)
    ei = rng.integers(0, N_NODES, size=(2, N_EDGES)).astype(np.int64)
    ew = rng.random(N_EDGES).astype(np.float32)
    W1 = rng.standard_normal((F_IN, F_HID)).astype(np.float32) * 0.09
    W2 = rng.standard_normal((F_HID, F_OUT)).astype(np.float32) * 0.09
    y = kernel(x, ei, ew, W1, np.zeros(F_HID, np.float32), W2, np.zeros(F_OUT, np.float32))
    print(y.shape, y.dtype, np.isfinite(y).all())
